# revision 43
# baseline (speedup 1.0000x reference)
"""Trainium2 Bass kernel for nn_MemLayer (retrieval_knn).

Math:  out[b,o] = -mean_d (x[b,d] - w[o,d])^2 + bias[o]
              =  s * (x' @ w'.T)[b,o]  -  ||x_b||^2/D  +  (bias[o] - ||w_o||^2/D)

  with x' = 16*x, w' = 4096*w in fp8e4m3 and s = 2/(D*16*4096). The GEMM term
  is computed on device in fp8 (scaled by 2^15 into the e4m3 range); the exact
  rank-1 corrections are applied on the host in fp32 (rel err ~3e-4, far
  inside the gate).

Strategy:
  - Data-parallel shard x along batch across 8 NeuronCores (1024 rows each),
    replicate weights. No cross-core communication; gather on host.
  - Per core: fp8 GEMM [1024,1024] @ [1024,4096] with DoubleRow perf mode
    (contraction 256 per matmul -> 256 matmuls of FD=512; measured steady
    issue rate 216ns/matmul at 2.4GHz = the FD-cycle streaming limit).
  - Head: warmup matmuls on a scratch tile start as the first tensor work
    and are sized (NWARM) so they end right when the input delivery curve
    can sustain real matmuls (~12.3-13.9us); they keep the PE HAM activity
    counter busy so the 1.2->2.4GHz unthrottle fires before (or right
    after) real matmuls start, and real matmuls follow seamlessly. Input
    pieces are ordered by first-use time: sync ring carries wk (kc-split
    128KB pieces then whole n-tiles), scalar ring carries xk as (half, kc)
    128KB pieces each covering the full 512-column half, so the first real
    matmul is gated on only wk-kc0 + xk-kc0 (128KB each) and later groups'
    pieces arrive just ahead of their consumption.
  - Loop: nt (n-tile) outer so the 4MB weight stream trickles in; per nt,
    four groups of 2 m-tiles. Within a group, m-tile j=0 accumulates its 4
    kc matmuls back-to-back into a 1-bank PSUM tile, then j=1 (j-outer order
    lets bank A's eviction start half a group early). Bank A evicts via
    Scalar ACT, bank B via DVE tensor-scalar - parallel 1-bank evictions
    (~0.65/0.83us) into a per-nt fp8 staging tile, so both engines sit at
    ~40-48% duty and PSUM slots recycle quickly (6x 1-bank rotation).
  - Output: one batched DMA per nt ([128, 4KB-contiguous] descriptors) on
    the Scalar HWDGE ring - 8 descriptor-generation triggers instead of 32,
    and 4KB descriptors instead of 1KB, so the out ring never backlogs.
  - Tail: the final n-tile's prefix (3 groups) goes out as soon as its
    evictions finish; the last group's two banks evict on ACT and DVE in
    parallel and drain as two 64KB pieces on the idle Sync + Scalar rings.
  - Host applies the exact rank-1 corrections and unshuffles the layout.
"""

import numpy as np
import ml_dtypes

B, D, O = 8192, 1024, 4096
NCORES = 8
BL = B // NCORES     # 1024 rows per core
P = 128
MT = BL // P         # 8 m-tiles
NTILE = 512          # one PSUM bank of fp32
NT = O // NTILE      # 8 n-tiles
GRP = 2              # m-tiles per eviction group

KD = D // (2 * P)    # 4 double-k-tiles (fp8 DoubleRow)
XSCALE = 16.0        # x -> fp8 pre-scale
WSCALE = 4096.0      # w -> fp8 pre-scale
OUT_SCALE = 32768.0  # fp8 output post-scale (divided out on host)

NWARM = 42           # warmup matmuls (FD=128, ~110-133ns each cold)

_CACHE = {}


def _get_nc():
    key = "nc_v4"
    if key in _CACHE:
        return _CACHE[key]

    import concourse.bacc as bacc
    import concourse.tile as tile
    from concourse import mybir

    nc = bacc.Bacc("TRN2", target_bir_lowering=False)

    f32 = mybir.dt.float32
    fp8 = mybir.dt.float8e4

    # xk[h, p, kc, i, c] = x'[kc*256 + i*128 + p, h*512 + c]
    xk_d = nc.dram_tensor("xk", [2, P, KD, 2, BL // 2], fp8,
                          kind="ExternalInput")
    wk_d = nc.dram_tensor("wk", [NT, P, KD, 2, NTILE], fp8, kind="ExternalInput")
    out_d = nc.dram_tensor("out", [P, NT, (MT // GRP) * GRP * NTILE], fp8,
                           kind="ExternalOutput")

    act_scale = float(2.0 / (D * XSCALE * WSCALE) * OUT_SCALE)
    DR = mybir.MatmulPerfMode.DoubleRow
    Ident = mybir.ActivationFunctionType.Identity

    with tile.TileContext(nc) as tc:
        with (
            tc.tile_pool(name="const", bufs=1) as cpool,
            tc.tile_pool(name="psum", bufs=6, space="PSUM") as ppool,
            tc.tile_pool(name="outp", bufs=2) as opool,
        ):
            xk_sb = cpool.tile([P, 2, KD, 2, BL // 2], fp8)
            wk_sb = cpool.tile([P, NT, KD, 2, NTILE], fp8)

            # Warmup operand (Tile requires a writer before reads; gpsimd's
            # engine start is measurably the earliest memset path - vector's
            # main starts ~1us later, scalar has no memset). Only half the
            # tile needs writing to satisfy the allocator; the unwritten
            # half is garbage, which a warmup operand doesn't care about.
            # The warmup start is gated by gpsimd engine readiness
            # (7.0-7.9us run-to-run), not by the memset size. The PSUM slot
            # is overwritten by a later start=True accumulation.
            junk = cpool.tile([P, 2, P], fp8)
            nc.gpsimd.memset(junk[:, 0], 0.0)

            # Input pieces in first-use order on the two HWDGE rings.
            # Sync: weights. First piece (kc0 of nt0, 128KB) gates the first
            # real matmul; remaining kc pieces follow, then whole n-tiles
            # (4KB/partition descriptors).
            # wk n-tile 0 goes kc-granular (128KB, 1KB descriptors - its
            # deadlines are the tightest); wk n-tile 1 goes in kc-pairs
            # (256KB, 2KB descriptors - half the per-byte descriptor
            # overhead, and nt1's first groups still start on the kc01
            # piece while kc23 is in flight).
            # kc0 and kc1 stay solo (tightest gates); kc2+kc3 pair up -
            # their deadlines are only 0.2us apart, the pair saves one
            # ~0.7us serial DIRECT2D generation slot on the sequencer and
            # doubles the descriptor size.
            nc.sync.dma_start(out=wk_sb[:, 0, 0], in_=wk_d[0, :, 0])
            nc.sync.dma_start(out=wk_sb[:, 0, 1], in_=wk_d[0, :, 1])
            nc.sync.dma_start(out=wk_sb[:, 0, 2:KD], in_=wk_d[0, :, 2:KD])
            nc.sync.dma_start(out=wk_sb[:, 1, 0:2], in_=wk_d[1, :, 0:2])
            nc.sync.dma_start(out=wk_sb[:, 1, 2:KD], in_=wk_d[1, :, 2:KD])

            # Scalar: x in (half, kc) pieces of 128KB, each covering the
            # full 512-column half so a piece serves both groups of that
            # half. Ordered by first consumption: half0 kc0..3 (group 0/1),
            # then half1 kc0..3 (groups 2/3 at +3.5/+5.2us). The head is
            # aggregate-bandwidth-bound (~250GB/s across all queues with the
            # SEngine 2:1 mux partner running the same schedule), so the
            # warmup count above is sized to start real matmuls right when
            # the delivery curve can sustain them - a third (SWDGE) queue
            # or an earlier start just moves the stall around (measured).
            nc.scalar.dma_start(out=xk_sb[:, 0, 0], in_=xk_d[0, :, 0])
            nc.scalar.dma_start(out=xk_sb[:, 0, 1], in_=xk_d[0, :, 1])
            nc.scalar.dma_start(out=xk_sb[:, 0, 2:KD], in_=xk_d[0, :, 2:KD])
            # half1 (first needed at +3.5us) in kc-pairs: 2KB descriptors.
            nc.scalar.dma_start(out=xk_sb[:, 1, 0:2], in_=xk_d[1, :, 0:2])
            nc.scalar.dma_start(out=xk_sb[:, 1, 2:KD], in_=xk_d[1, :, 2:KD])

            # Warmup: FD=128 matmuls on the junk tile keep the PE HAM
            # activity counter running while the input DMA head is in
            # flight, so the 1.2->2.4GHz unthrottle fires early.
            ps_warm = ppool.tile([P, NTILE], f32, tag="ps")
            for w in range(NWARM):
                nc.tensor.matmul(
                    ps_warm[:, 0:P],
                    lhsT=junk[:],
                    rhs=junk[:],
                    start=True,
                    stop=True,
                    perf_mode=DR,
                )

            for nt in range(NT):
                if nt + 2 < NT:
                    nc.sync.dma_start(out=wk_sb[:, nt + 2], in_=wk_d[nt + 2])
                last_nt = nt == NT - 1
                obs = opool.tile([P, MT * NTILE], fp8, tag="obs")
                for q in range(MT // GRP):
                    banks = []
                    for j in range(GRP):
                        ps = ppool.tile([P, NTILE], f32, tag="ps")
                        banks.append(ps)
                        mt = q * GRP + j
                        half = mt // (MT // 2)
                        c0 = (mt % (MT // 2)) * P
                        for kc in range(KD):
                            nc.tensor.matmul(
                                ps[:],
                                lhsT=xk_sb[:, half, kc, :, c0:c0 + P],
                                rhs=wk_sb[:, nt, kc, :, :],
                                start=(kc == 0),
                                stop=(kc == KD - 1),
                                perf_mode=DR,
                            )
                    o0 = q * GRP * NTILE
                    # Parallel 1-bank evictions: bank A on ACT (scalar),
                    # bank B on DVE (vector).
                    nc.scalar.activation(obs[:, o0:o0 + NTILE], banks[0][:],
                                         Ident, scale=act_scale)
                    nc.vector.tensor_scalar_mul(obs[:, o0 + NTILE:o0 + 2 * NTILE],
                                                banks[1][:], act_scale)
                    if last_nt and q == MT // GRP - 2:
                        # Final n-tile prefix drains early so only the last
                        # group's two 64KB pieces trail the last matmul.
                        nc.scalar.dma_start(out=out_d[:, nt, 0:3 * GRP * NTILE],
                                            in_=obs[:, 0:3 * GRP * NTILE])
                if last_nt:
                    nc.sync.dma_start(out=out_d[:, nt, 3 * GRP * NTILE:
                                                7 * NTILE],
                                      in_=obs[:, 3 * GRP * NTILE:7 * NTILE])
                    nc.scalar.dma_start(out=out_d[:, nt, 7 * NTILE:],
                                        in_=obs[:, 7 * NTILE:])
                else:
                    nc.scalar.dma_start(out=out_d[:, nt, :], in_=obs[:])

    nc.finalize()
    _CACHE[key] = nc
    return nc


def _prep_inputs(x, weights, bias):
    """Shard + lay out host inputs -> per-core in_maps (+ host corrections)."""
    x = np.asarray(x, dtype=np.float32)
    weights = np.asarray(weights, dtype=np.float32)
    bias = np.asarray(bias, dtype=np.float32)

    dt = ml_dtypes.float8_e4m3
    # k = kd*256 + i*128 + p
    wT = weights.T * np.float32(WSCALE)                       # [D, O]
    wk = np.ascontiguousarray(
        wT.reshape(KD, 2, P, NT, NTILE)
        .transpose(3, 2, 0, 1, 4)
        .astype(dt)
    )

    in_maps = []
    for c in range(NCORES):
        xs = x[c * BL:(c + 1) * BL]                            # [BL, D] fp32
        xT = xs.T                                              # [D, BL]
        # xk[h, p, kc, i, c] = x'[kc*256 + i*128 + p, h*512 + c]
        xk = np.ascontiguousarray(
            (xT.reshape(KD, 2, P, 2, BL // 2) * np.float32(XSCALE))
            .transpose(3, 2, 0, 1, 4)
            .astype(dt)
        )
        in_maps.append({"xk": xk, "wk": wk})

    # Host-side rank-1 corrections (exact fp32)
    w_sq = np.einsum("od,od->o", weights, weights)
    _CACHE["v"] = (bias - w_sq / np.float32(D)).astype(np.float32)     # [O]
    _CACHE["xsq"] = (-np.einsum("bd,bd->b", x, x) / np.float32(D)
                     ).astype(np.float32)                              # [B]
    return in_maps


def _gather(results):
    parts = []
    for c in range(NCORES):
        o = np.asarray(results[c]["out"])        # [P, NT, (MT//GRP)*GRP*NTILE]
        o = o.reshape(P, NT, MT // GRP, GRP, NTILE)
        # b_local = (q*GRP + j)*P + p ; o_col = nt*NTILE + col
        o = o.transpose(2, 3, 0, 1, 4).reshape(BL, O)
        parts.append(o)
    full = np.concatenate(parts, axis=0).astype(np.float32)
    full *= np.float32(1.0 / OUT_SCALE)
    full += _CACHE["xsq"][:, None]
    full += _CACHE["v"][None, :]
    return np.ascontiguousarray(full)


def _run(in_maps, **kwargs):
    from concourse.bass_utils import run_bass_kernel_spmd

    nc = _get_nc()
    return run_bass_kernel_spmd(nc, in_maps, core_ids=list(range(NCORES)), **kwargs)


def kernel(x, weights, bias):
    in_maps = _prep_inputs(x, weights, bias)
    res = _run(in_maps)
    return _gather(res.results)
